# revision 5
# baseline (speedup 1.0000x reference)
"""Multi-head linear attention (Performer/FAVOR+) Bass kernel for 8x TRN2 cores.

Sharding: 8 cores = 4 batches x 2 head-groups. Core c handles batch c//2 and
heads [4*(c%2), 4*(c%2)+4).

Math notes (exact rewrites of the reference, not approximations):
  - omega is sqrt(64) * orthogonal, so Omega @ Omega.T = 64*I. Hence
    0.5*||q||^2 = ||q @ Omega.T||^2 / 128: the squared-sum term is computed
    from xw itself and the plain q/k projections are never needed.
  - The per-row scale exp(-sq_t) on phi(q), the global 1/sqrt(128) scale and
    (approximately) the +EPS term all cancel in out = qkv[..,:64]/qkv[..,64],
    so the q-side feature map is just exp(+-xw).
  - The k-side scale rho_s = exp(-ksq_s) is folded into v1 = [v, 1]*rho so
    kp is also just exp(+-kxw).
  - The final divide (qkv[..,:64] / qkv[..,64]) runs on the HOST: the NEFF
    streams out raw [qkv_v | normalizer] per head in bf16.

Q-side projection computes only the 64 positive features per head (wqp2
packs two heads into the 128 stationary columns); exp(+x) and exp(-x) are
two full-partition ACT ops over the same PSUM tile, giving qpP=[h0+,h1+]
and qpM=[h0-,h1-] tiles. The qkv matmuls then use block-diagonal moving
operands kvBD_P/kvBD_M (built once per rep with cross-partition-offset ACT
copies) so one 128-contraction MM yields both heads' partial products.

Layouts: inputs are pre-transposed to f-major fp16 on the host. All
projections contract f=512 over 4 chunks of 128 partitions. PSUM fp32.

Schedule: phase-Q projection work (qxw matmuls + exp) is interleaved into the
KV s-loop to fill dependency stalls; the qkv raw-dump tail runs last.
PSUM budget (8 banks): kxwv(2x2) + qx2(2) + kvacc(1); tail qkv units reuse
the kxwv/qx2 tags (2 banks each, 3 rotating).
"""

import sys

import numpy as np

for _p in ("/opt/trn_rl_repo", "/root/.axon_site/_ro/trn_rl_repo"):
    try:
        import concourse  # noqa: F401
        break
    except ImportError:
        if _p not in sys.path:
            sys.path.insert(0, _p)

B, T, D, H = 4, 4096, 512, 8
DK = DV = 64
HPC = 4            # heads per core
NCH = 4            # f chunks (512 / 128)
P = 128
ST = T // P        # 32 s-tiles
TC = 8             # t chunks
TCW = T // TC      # 512

_CACHE = {}


def _build_program(reps=1):
    import concourse.mybir as mybir
    import concourse.tile as tile
    from concourse import bacc
    from contextlib import ExitStack

    dt = mybir.dt
    AF = mybir.ActivationFunctionType

    nc = bacc.Bacc("TRN2", target_bir_lowering=False, debug=False)

    qt_d = nc.dram_tensor("qt", [D, T], dt.float16, kind="ExternalInput")
    kt_d = nc.dram_tensor("kt", [D, T], dt.float16, kind="ExternalInput")
    vt_d = nc.dram_tensor("vt", [D, T], dt.float16, kind="ExternalInput")
    wqp2_d = nc.dram_tensor("wqp2", [2, NCH, P, P], dt.float16, kind="ExternalInput")
    wko_d = nc.dram_tensor("wko", [NCH, P, HPC * DK], dt.float16, kind="ExternalInput")
    wv_d = nc.dram_tensor("wv", [NCH, P, HPC * DV], dt.float16, kind="ExternalInput")
    # raw out: per t row, 2 head-pairs x 130 = [h_even v(64) | h_even n |
    # h_odd v(64) | h_odd n]
    out_d = nc.dram_tensor("out", [T * 2 * 130], dt.bfloat16, kind="ExternalOutput")

    with tile.TileContext(nc) as tc, ExitStack() as ctx:
        const = ctx.enter_context(tc.tile_pool(name="const", bufs=1))
        work = ctx.enter_context(tc.tile_pool(name="work", bufs=3))
        psum = ctx.enter_context(tc.tile_pool(name="psum", bufs=1, space="PSUM"))
        for _rep in range(reps):
            _emit_body(nc, tc, const, work, psum, mybir, dt, AF,
                       qt_d, kt_d, vt_d, wqp2_d, wko_d, wv_d, out_d)

    nc.compile()
    return nc


def _emit_body(nc, tc, const, work, psum, mybir, dt, AF,
               qt_d, kt_d, vt_d, wqp2_d, wko_d, wv_d, out_d):
    if True:

        # persistent SBUF residents
        qt = const.tile([P, NCH, T], dt.float16)
        kt = const.tile([P, NCH, T], dt.float16)
        vt = const.tile([P, NCH, T], dt.float16)
        wqp2 = const.tile([P, 2, NCH, P], dt.float16)
        wko = const.tile([P, NCH, HPC * DK], dt.float16)
        wv = const.tile([P, NCH, HPC * DV], dt.float16)
        kvbd_p = const.tile([P, 2, 130], dt.bfloat16)
        kvbd_m = const.tile([P, 2, 130], dt.bfloat16)

        # Coalesced loads: one DMA per (tensor, column window) spanning all
        # 4 f-chunks. Order matters: k/v first columns unblock pair 0,
        # wqp2/qt follow for the first q-chunk, then windows stream in the
        # order the loop consumes them.
        nc.sync.dma_start(out=wko[:], in_=wko_d.ap().rearrange("c p n -> p c n"))
        nc.sync.dma_start(out=wv[:], in_=wv_d.ap().rearrange("c p n -> p c n"))

        def load_win(dst, src_d, lo, hi):
            nc.sync.dma_start(
                out=dst[:, :, lo:hi],
                in_=src_d.ap()[:, lo:hi].rearrange("(c p) w -> p c w", p=P),
            )

        load_win(kt, kt_d, 0, 256)
        load_win(vt, vt_d, 0, 256)
        nc.sync.dma_start(out=wqp2[:],
                          in_=wqp2_d.ap().rearrange("g c p m -> p g c m"))
        load_win(qt, qt_d, 0, 512)
        load_win(kt, kt_d, 256, 512)
        load_win(vt, vt_d, 256, 512)
        for lo, hi in ((512, 1536), (1536, 2560), (2560, 3584), (3584, T)):
            load_win(kt, kt_d, lo, hi)
            load_win(vt, vt_d, lo, hi)
            load_win(qt, qt_d, lo, hi)

        nc.vector.memset(kvbd_p[:], 0.0)
        nc.vector.memset(kvbd_m[:], 0.0)

        # Single-bank PSUM accumulator: kv[h] at columns [h*65, h*65+65).
        # first_mm clears has_written at BANK granularity, so interleaved
        # per-head groups must NOT use start=True: memset the bank once and
        # accumulate from the first matmul.
        kv_big = psum.tile([P, HPC, DV + 1], dt.float32, tag="kvacc", bufs=1)
        nc.vector.memset(kv_big[:], 0.0)

        qp_tiles = []

        def emit_q_chunk(tcx):
            tsl = slice(tcx * TCW, (tcx + 1) * TCW)
            qx2 = psum.tile([P, 2, TCW], dt.float32, tag="qx2", bufs=1,
                            name="qx2")
            for hp in range(2):
                for c in range(NCH):
                    nc.tensor.matmul(
                        qx2[:, hp, :], wqp2[:, hp, c, :], qt[:, c, tsl],
                        start=(c == 0), stop=(c == NCH - 1),
                    )
            qpp = work.tile([P, 2, TCW], dt.bfloat16, tag="qpp", bufs=TC,
                            name=f"qpp{tcx}")
            qpm = work.tile([P, 2, TCW], dt.bfloat16, tag="qpm", bufs=TC,
                            name=f"qpm{tcx}")
            nc.scalar.activation(qpp[:], qx2[:], AF.Exp, scale=1.0)
            nc.scalar.activation(qpm[:], qx2[:], AF.Exp, scale=-1.0)
            qp_tiles.append((qpp, qpm))

        # ---------------- phase KV (with q-projection work interleaved) -----
        # Engines execute their queues IN ORDER, so cross-engine dependencies
        # are software-pipelined: v1 (needs rho from ACT) is emitted one pair
        # late on DVE, the kv matmuls (need v1) one pair later still on PE.
        NP_ = ST // 2    # 16 pairs
        stage = {}       # pi -> dict of tiles

        def emit_v1(pi):
            st_ = stage[pi]
            v1 = work.tile([P, 2, HPC, DV + 1], dt.bfloat16, tag="v1",
                           name="v1")
            nc.vector.tensor_mul(
                v1[:, :, :, 0:DV], st_["v_ps"],
                st_["rho"][:].broadcast_to([P, 2, HPC, DV])
            )
            nc.vector.tensor_copy(v1[:, :, :, DV:DV + 1], st_["rho"][:])
            st_["v1"] = v1

        def emit_kv(pi):
            st_ = stage.pop(pi)
            for p_ in range(2):
                si = 2 * pi + p_
                for h in range(HPC):
                    nc.tensor.matmul(
                        kv_big[:, h, :], st_["kp"][:, p_, h, :],
                        st_["v1"][:, p_, h, :],
                        start=False, stop=(si == ST - 1),
                        skip_group_check=True,
                    )

        for pi in range(NP_):
            # kxw and v share PSUM banks: [..., 0:64] = kxw, 64:128 = v
            kxwv = psum.tile([P, 2, HPC, 2 * DK], dt.float32, tag="kxwv",
                             bufs=2, name="kxwv")
            kxw = kxwv[:, :, :, 0:DK]
            v_ps = kxwv[:, :, :, DK:2 * DK]
            for p_ in range(2):
                ssl = slice((2 * pi + p_) * P, (2 * pi + p_ + 1) * P)
                for c in range(NCH):
                    nc.tensor.matmul(
                        kxwv[:, p_, :, 0:DK], kt[:, c, ssl], wko[:, c, :],
                        start=(c == 0), stop=(c == NCH - 1),
                    )
                for c in range(NCH):
                    nc.tensor.matmul(
                        kxwv[:, p_, :, DK:2 * DK], vt[:, c, ssl], wv[:, c, :],
                        start=(c == 0), stop=(c == NCH - 1),
                    )
            if pi >= 2:
                emit_kv(pi - 2)

            kp = work.tile([P, 2, HPC, 2 * DK], dt.bfloat16, tag="kp", bufs=3)
            nc.scalar.activation(kp[:, :, :, 0:DK], kxw, AF.Exp, scale=1.0)
            nc.scalar.activation(kp[:, :, :, DK:2 * DK], kxw, AF.Exp,
                                 scale=-1.0)

            kxw_sb = work.tile([P, 2, HPC, DK], dt.bfloat16, tag="kxwsb",
                               bufs=2)
            nc.vector.tensor_copy(kxw_sb[:], kxw)
            sqsc = work.tile([P, 2, HPC, DK], dt.bfloat16, tag="sqsc", bufs=2)
            nc.vector.tensor_mul(sqsc[:], kxw_sb[:], kxw_sb[:])
            ksqr = work.tile([P, 2, HPC, 1], dt.float32, tag="ksqr")
            nc.vector.reduce_sum(ksqr[:], sqsc[:], axis=mybir.AxisListType.X)
            rho = work.tile([P, 2, HPC, 1], dt.float32, tag="rho")
            nc.scalar.activation(rho[:], ksqr[:], AF.Exp, scale=-1.0 / 128.0)

            stage[pi] = {"v_ps": v_ps, "rho": rho, "kp": kp}
            if pi >= 1:
                emit_v1(pi - 1)

            if pi % 2 == 1:
                emit_q_chunk(pi // 2)

        emit_v1(NP_ - 1)
        emit_kv(NP_ - 2)
        emit_kv(NP_ - 1)

        # Block-diagonal kv for the 2-head qkv matmuls. kv_big rows: 0:64 =
        # plus feats, 64:128 = minus feats; columns per head. ACT copies
        # support cross-partition-offset placement.
        for hp in range(2):
            nc.scalar.copy(kvbd_p[0:64, hp, 0:65], kv_big[0:64, 2 * hp, :])
            nc.scalar.copy(kvbd_p[64:128, hp, 65:130],
                           kv_big[0:64, 2 * hp + 1, :])
            nc.scalar.copy(kvbd_m[0:64, hp, 0:65], kv_big[64:128, 2 * hp, :])
            nc.scalar.copy(kvbd_m[64:128, hp, 65:130],
                           kv_big[64:128, 2 * hp + 1, :])

        # ---------------- tail: qkv raw dump (normalize on host) ------------
        # Units of 2 t-tiles: psum [128, 2, 2, 130] = 2 banks, rotating
        # through the kxwv(x2) + qx2 tags.
        unit_tags = [("kxwv", 2), ("kxwv", 2), ("qx2", 1)]
        ui = 0
        for tcx in range(TC):
            qpp, qpm = qp_tiles[tcx]
            for u in range(2):
                tg, bufs_ = unit_tags[ui % 3]
                ui += 1
                # regions padded to 256 fp32 so none crosses a PSUM bank
                # boundary (start=True clears has_written only for the bank
                # containing the write start; a region spilling into the
                # next bank would accumulate onto stale data on buf reuse)
                qkv = psum.tile([P, 2, 2, 256], dt.float32, tag=tg,
                                bufs=bufs_, name="qkv")
                for tt2 in range(2):
                    ttsl = slice((u * 2 + tt2) * P, (u * 2 + tt2 + 1) * P)
                    for hp in range(2):
                        nc.tensor.matmul(
                            qkv[:, tt2, hp, 0:130], qpp[:, hp, ttsl],
                            kvbd_p[:, hp, :], start=True, stop=False,
                        )
                        nc.tensor.matmul(
                            qkv[:, tt2, hp, 0:130], qpm[:, hp, ttsl],
                            kvbd_m[:, hp, :], start=False, stop=True,
                        )
                o_sb = work.tile([P, 2, 2, 130], dt.bfloat16, tag="osb",
                                 bufs=3)
                nc.vector.tensor_copy(o_sb[:], qkv[:, :, :, 0:130])
                base = (tcx * 512 + u * 256) * 260
                nc.gpsimd.dma_start(
                    out=out_d.ap()[base:base + 256 * 260].rearrange(
                        "(tt2 p c) -> p tt2 c", tt2=2, p=P
                    ),
                    in_=o_sb[:],
                )


def _get_program(reps=1):
    if reps not in _CACHE:
        _CACHE[reps] = _build_program(reps)
    return _CACHE[reps]


def _prep_core_inputs(query, value, key, wqo, wko, wv_w, core):
    b, hg = core // 2, core % 2
    hs = slice(hg * HPC, (hg + 1) * HPC)

    qT = np.ascontiguousarray(query[b].T.astype(np.float16))   # (512, 4096)
    kT = np.ascontiguousarray(key[b].T.astype(np.float16))
    vT = np.ascontiguousarray(value[b].T.astype(np.float16))

    wqo_c = wqo[hs]                                            # (4, 512, 64)
    # head-pair packing: [h_even 64 cols | h_odd 64 cols]
    wqp2 = np.stack([
        np.concatenate([wqo_c[2 * hp], wqo_c[2 * hp + 1]], axis=1)
        for hp in range(2)
    ])                                                         # (2, 512, 128)
    wqp2 = np.ascontiguousarray(
        wqp2.reshape(2, NCH, P, P).astype(np.float16))         # (hp, c, p, m)

    wko_c = np.concatenate(list(wko[hs]), axis=1)              # (512, 256)
    wko_c = np.ascontiguousarray(
        wko_c.reshape(NCH, P, HPC * DK).astype(np.float16))
    wv_c = np.concatenate(list(wv_w[hs]), axis=1)              # (512, 256)
    wv_c = np.ascontiguousarray(
        wv_c.reshape(NCH, P, HPC * DV).astype(np.float16))

    return {"qt": qT, "kt": kT, "vt": vT,
            "wqp2": wqp2, "wko": wko_c, "wv": wv_c}


def kernel(query, value, key, wq, wv, wk, omega):
    from concourse.bass_utils import run_bass_kernel_spmd

    query = np.asarray(query, np.float32)
    value = np.asarray(value, np.float32)
    key = np.asarray(key, np.float32)
    wq = np.asarray(wq, np.float32)
    wv = np.asarray(wv, np.float32)
    wk = np.asarray(wk, np.float32)
    omega = np.asarray(omega, np.float32)

    nc = _get_program()

    wqo = np.einsum("hfk,mk->hfm", wq, omega)                  # (8, 512, 64)
    wko = np.einsum("hfk,mk->hfm", wk, omega)

    in_maps = [
        _prep_core_inputs(query, value, key, wqo, wko, wv, core)
        for core in range(8)
    ]
    res = run_bass_kernel_spmd(nc, in_maps, core_ids=list(range(8)))

    out = np.empty((B, T, D), np.float32)
    for core in range(8):
        b, hg = core // 2, core % 2
        raw = np.asarray(res.results[core]["out"], np.float32)
        raw = raw.reshape(T, 2, 130)                           # (t, hp, 130)
        # head h_local = 2*hp + j lives at cols [j*65, j*65+65)
        ov = np.empty((HPC, T, DV), np.float32)
        for hl in range(HPC):
            hp, j = hl // 2, hl % 2
            blk = raw[:, hp, j * 65:(j + 1) * 65]
            ov[hl] = blk[:, 0:DV] / blk[:, DV:DV + 1]
        out[b, hg * 2048:(hg + 1) * 2048, :] = ov.reshape(2048, 512)
    return out
